# revision 12
# baseline (speedup 1.0000x reference)
"""Trainium2 Bass kernel for AdaptiveEmbeddingGraphBuilder.

Computes out = row_softmax(topk_mask(relu(E @ E.T), k=10)) for E [8192, 64],
row-sharded across 8 NeuronCores (1024 rows each).

Device side (per core, per 128-row block of A = E_rows @ E_full^T):
  - PE: plain fp8(e4m3) matmuls, K=64, into eight 1024-wide PSUM regions
    covering the 8192 columns.  Measured: the PE streams 512 moving rows
    per matmul at a fixed ~427 ns (1.2 GHz, no pstate ramp, and none of
    the fp8 perf modes change it), so PE time is fixed at ~55 us/core and
    fp8 only helps by shrinking input DMA and SBUF.  fp8 dot noise (~0.3)
    is irrelevant for window *ranking* (margins are >10) and the host
    recomputes exact values.
  - ACT: converts regions 0..3 (cols 0..4095) to an fp16 SBUF tile A16.
  - DVE: regions 4..7 are consumed by fused scalar_tensor_tensor ops
    (single PSUM input each, as required by the ISA):
      pooled[:, c] = max(psum[:, j], A16[:, c]) = max(A[:, c], A[:, c+4096])
    writing fp8 directly (6% value noise, fine for ranking).
  - DMA out pooled [128, 4096] fp8 per block (4 MB/core total).

Host side: per row take the top-16 pooled 2-column windows (any column
with value >= v10 lands in a window whose pooled value is >= v10, and at
most 10 windows can satisfy that, so top-16 always contains the true
top-10); recompute the 32 candidate dots exactly in fp64, take the exact
top-10, and emit the exact masked softmax (kept entries exp(v-m)/D,
dropped entries exp(-m)/D with D = sum exp(v_k-m) + (N-10) exp(-m)).
"""

import numpy as np

N = 8192
D = 64
K = 10
NCORES = 8
P = 128
REG = 1024  # PSUM region width (2 banks)
NREG = 8
MM = 512  # single-matmul moving width
NACT = 4  # regions converted by ACT; the rest are folded by DVE from PSUM
ROWS_PER_CORE = N // NCORES  # 1024
NBLOCKS = ROWS_PER_CORE // P  # 8
ACCW = 4096  # pooled output width per row
OUT_F8 = True  # pooled output dtype fp8e4 (else fp16)
GP_REGIONS = 0  # how many of the DVE regions to offload to GpSimd


def build(n=N, rows_per_core=ROWS_PER_CORE, out_f8=OUT_F8, gp_regions=GP_REGIONS):
    import concourse.bacc as bacc
    import concourse.mybir as mybir
    import concourse.tile as tile

    nblocks = rows_per_core // P
    f32 = mybir.dt.float32
    f16 = mybir.dt.float16
    f8 = mybir.dt.float8e4
    odt = f8 if out_f8 else f16
    Copy = mybir.ActivationFunctionType.Copy
    Max = mybir.AluOpType.max
    nc = bacc.Bacc("TRN2", target_bir_lowering=False, debug=False)
    et_d = nc.declare_dram_parameter("et", [D, n], f8, isOutput=False)
    lhs_d = nc.declare_dram_parameter("lhs", [D, rows_per_core], f8, isOutput=False)
    out_d = nc.declare_dram_parameter("out", [rows_per_core, ACCW], odt, isOutput=True)

    with tile.TileContext(nc) as tc:
        with (
            tc.tile_pool(name="const", bufs=1) as cpool,
            tc.tile_pool(name="acc", bufs=2) as apool,
            tc.tile_pool(name="outp", bufs=2) as opool,
            tc.tile_pool(name="psum", bufs=4, space="PSUM") as ppool,
        ):
            lhs_sb = cpool.tile([D, rows_per_core], f8)
            et_sb = cpool.tile([D, n], f8)
            nc.sync.dma_start(out=lhs_sb[:], in_=lhs_d[:])
            nc.sync.dma_start(out=et_sb[:], in_=et_d[:])

            def region_matmuls(dst, b, r):
                for c in range(REG // MM):
                    lo = r * REG + c * MM
                    nc.tensor.matmul(
                        out=dst[:, c * MM : (c + 1) * MM],
                        lhsT=lhs_sb[:, b * P : (b + 1) * P],
                        rhs=et_sb[:, lo : lo + MM],
                        start=True,
                        stop=True,
                    )

            for b in range(nblocks):
                A16 = apool.tile([P, NACT * REG], f16, tag="A")
                for r in range(NACT):
                    pr = ppool.tile([P, REG], f32, tag="ps")
                    region_matmuls(pr, b, r)
                    nc.scalar.activation(
                        out=A16[:, r * REG : (r + 1) * REG], in_=pr[:], func=Copy
                    )
                tblk = opool.tile([P, ACCW], odt, tag="T")
                for i, r in enumerate(range(NACT, NREG)):
                    pr = ppool.tile([P, REG], f32, tag="ps")
                    region_matmuls(pr, b, r)
                    eng = nc.gpsimd if i >= (NREG - NACT) - gp_regions else nc.vector
                    # fused PSUM read + fold with an ACT strip (one PSUM input)
                    eng.scalar_tensor_tensor(
                        out=tblk[:, i * REG : (i + 1) * REG],
                        in0=pr[:],
                        scalar=-3.0e38,
                        in1=A16[:, i * REG : (i + 1) * REG],
                        op0=Max,
                        op1=Max,
                    )
                nc.sync.dma_start(out=out_d[b * P : (b + 1) * P, :], in_=tblk[:])
    nc.compile()
    return nc


def _prep_inputs(node_emb):
    """fp8 cast + transpose + row-shard. Returns per-core in_maps."""
    import ml_dtypes

    x = np.asarray(node_emb, dtype=np.float32)
    cat = x.astype(ml_dtypes.float8_e4m3)  # [n, 64]
    et = np.ascontiguousarray(cat.T)  # [64, n]
    in_maps = []
    for c in range(NCORES):
        lhs = np.ascontiguousarray(cat[c * ROWS_PER_CORE : (c + 1) * ROWS_PER_CORE].T)
        in_maps.append({"et": et, "lhs": lhs})
    return in_maps


def _host_finish(x, pooled):
    """Exact top-10 masked softmax from the pooled device output.

    x: [N, 64] fp32 node embeddings; pooled: [N, accw] with
    pooled[:, c] = max over t of A[:, c + accw*t].
    """
    Pv = pooled.astype(np.float32)
    n = Pv.shape[0]
    accw = Pv.shape[1]
    nw = N // accw  # window size (columns per window)
    w = np.argpartition(-Pv, 16, axis=1)[:, :16]  # [n,16] top-16 windows
    cand = (w[:, :, None] + accw * np.arange(nw)[None, None, :]).reshape(n, 16 * nw)
    X = x.astype(np.float64)
    V = np.einsum("nd,nkd->nk", X, X[cand])  # exact fp64 dots
    V = np.maximum(V, 0.0)
    top = np.argpartition(-V, K, axis=1)[:, :K]
    rows = np.arange(n)[:, None]
    v = V[rows, top]
    cols = cand[rows, top]
    m = v.max(axis=1, keepdims=True)
    ex = np.exp(v - m)
    Dm = ex.sum(axis=1, keepdims=True) + (N - K) * np.exp(-m)
    base = (np.exp(-m) / Dm).astype(np.float32)
    kept = (ex / Dm).astype(np.float32)
    out = np.empty((n, N), np.float32)
    out[:] = base
    out[rows, cols] = kept
    return out


_CACHED_NC = None


def kernel(node_emb):
    global _CACHED_NC
    from concourse.bass_utils import run_bass_kernel_spmd

    if _CACHED_NC is None:
        _CACHED_NC = build()
    x = np.asarray(node_emb, dtype=np.float32)
    in_maps = _prep_inputs(x)
    res = run_bass_kernel_spmd(_CACHED_NC, in_maps, core_ids=list(range(NCORES)))
    pooled = np.concatenate([res.results[c]["out"] for c in range(NCORES)], axis=0)
    return _host_finish(x, pooled)


# revision 13
# speedup vs baseline: 1.0596x; 1.0596x over previous
"""Trainium2 Bass kernel for AdaptiveEmbeddingGraphBuilder.

Computes out = row_softmax(topk_mask(relu(E @ E.T), k=10)) for E [8192, 64],
row-sharded across 8 NeuronCores (1024 rows each).

Device side (per core, per 128-row block of A = E_rows @ E_full^T):
  - PE: plain fp8(e4m3) matmuls, K=64, into eight 1024-wide PSUM regions
    covering the 8192 columns.  Measured: the PE streams 512 moving rows
    per matmul at a fixed ~427 ns (1.2 GHz, no pstate ramp, and none of
    the fp8 perf modes change it), so PE time is fixed at ~55 us/core and
    fp8 only helps by shrinking input DMA and SBUF.  fp8 dot noise (~0.3)
    is irrelevant for window *ranking* (margins are >10) and the host
    recomputes exact values.
  - ACT: converts regions 0..3 (cols 0..4095) to an fp16 SBUF tile A16.
  - DVE: regions 4..7 are consumed by fused scalar_tensor_tensor ops
    (single PSUM input each, as required by the ISA):
      pooled[:, c] = max(psum[:, j], A16[:, c]) = max(A[:, c], A[:, c+4096])
    writing fp8 directly (6% value noise, fine for ranking).
  - DMA out pooled [128, 4096] fp8 per block (4 MB/core total).

Host side: per row take the top-16 pooled 2-column windows (any column
with value >= v10 lands in a window whose pooled value is >= v10, and at
most 10 windows can satisfy that, so top-16 always contains the true
top-10); recompute the 32 candidate dots exactly in fp64, take the exact
top-10, and emit the exact masked softmax (kept entries exp(v-m)/D,
dropped entries exp(-m)/D with D = sum exp(v_k-m) + (N-10) exp(-m)).
"""

import numpy as np

N = 8192
D = 64
K = 10
NCORES = 8
P = 128
REG = 1024  # PSUM region width (2 banks)
NREG = 8
MM = 512  # single-matmul moving width
NACT = 4  # regions converted by ACT; the rest are folded by DVE from PSUM
ROWS_PER_CORE = N // NCORES  # 1024
NBLOCKS = ROWS_PER_CORE // P  # 8
ACCW = 4096  # pooled output width per row
OUT_F8 = True  # pooled output dtype fp8e4 (else fp16)
GP_REGIONS = 0  # how many of the DVE regions to offload to GpSimd


def build(n=N, rows_per_core=ROWS_PER_CORE, out_f8=OUT_F8, gp_regions=GP_REGIONS):
    import concourse.bacc as bacc
    import concourse.mybir as mybir
    import concourse.tile as tile

    nblocks = rows_per_core // P
    f32 = mybir.dt.float32
    f16 = mybir.dt.float16
    f8 = mybir.dt.float8e4
    odt = f8 if out_f8 else f16
    Copy = mybir.ActivationFunctionType.Copy
    Max = mybir.AluOpType.max
    nc = bacc.Bacc("TRN2", target_bir_lowering=False, debug=False)
    et_d = nc.declare_dram_parameter("et", [D, n], f8, isOutput=False)
    lhs_d = nc.declare_dram_parameter("lhs", [D, rows_per_core], f8, isOutput=False)
    out_d = nc.declare_dram_parameter("out", [rows_per_core, ACCW], odt, isOutput=True)

    with tile.TileContext(nc) as tc:
        with (
            tc.tile_pool(name="const", bufs=1) as cpool,
            tc.tile_pool(name="acc", bufs=3) as apool,
            tc.tile_pool(name="outp", bufs=3) as opool,
            tc.tile_pool(name="psum", bufs=4, space="PSUM") as ppool,
        ):
            lhs_sb = cpool.tile([D, rows_per_core], f8)
            et_sb = cpool.tile([D, n], f8)
            nc.sync.dma_start(out=lhs_sb[:], in_=lhs_d[:])
            for r in range(NREG):
                nc.sync.dma_start(
                    out=et_sb[:, r * REG : (r + 1) * REG],
                    in_=et_d[:, r * REG : (r + 1) * REG],
                )

            def region_matmuls(dst, b, r):
                for c in range(REG // MM):
                    lo = r * REG + c * MM
                    nc.tensor.matmul(
                        out=dst[:, c * MM : (c + 1) * MM],
                        lhsT=lhs_sb[:, b * P : (b + 1) * P],
                        rhs=et_sb[:, lo : lo + MM],
                        start=True,
                        stop=True,
                    )

            for b in range(nblocks):
                A16 = apool.tile([P, NACT * REG], f16, tag="A")
                for r in range(NACT):
                    pr = ppool.tile([P, REG], f32, tag="ps")
                    region_matmuls(pr, b, r)
                    nc.scalar.activation(
                        out=A16[:, r * REG : (r + 1) * REG], in_=pr[:], func=Copy
                    )
                tblk = opool.tile([P, ACCW], odt, tag="T")
                for i, r in enumerate(range(NACT, NREG)):
                    pr = ppool.tile([P, REG], f32, tag="ps")
                    region_matmuls(pr, b, r)
                    eng = nc.gpsimd if i >= (NREG - NACT) - gp_regions else nc.vector
                    # fused PSUM read + fold with an ACT strip (one PSUM input)
                    eng.scalar_tensor_tensor(
                        out=tblk[:, i * REG : (i + 1) * REG],
                        in0=pr[:],
                        scalar=-3.0e38,
                        in1=A16[:, i * REG : (i + 1) * REG],
                        op0=Max,
                        op1=Max,
                    )
                nc.sync.dma_start(out=out_d[b * P : (b + 1) * P, :], in_=tblk[:])
    nc.compile()
    return nc


def _prep_inputs(node_emb):
    """fp8 cast + transpose + row-shard. Returns per-core in_maps."""
    import ml_dtypes

    x = np.asarray(node_emb, dtype=np.float32)
    cat = x.astype(ml_dtypes.float8_e4m3)  # [n, 64]
    et = np.ascontiguousarray(cat.T)  # [64, n]
    in_maps = []
    for c in range(NCORES):
        lhs = np.ascontiguousarray(cat[c * ROWS_PER_CORE : (c + 1) * ROWS_PER_CORE].T)
        in_maps.append({"et": et, "lhs": lhs})
    return in_maps


def _host_finish(x, pooled):
    """Exact top-10 masked softmax from the pooled device output.

    x: [N, 64] fp32 node embeddings; pooled: [N, accw] with
    pooled[:, c] = max over t of A[:, c + accw*t].
    """
    Pv = pooled.astype(np.float32)
    n = Pv.shape[0]
    accw = Pv.shape[1]
    nw = N // accw  # window size (columns per window)
    w = np.argpartition(-Pv, 16, axis=1)[:, :16]  # [n,16] top-16 windows
    cand = (w[:, :, None] + accw * np.arange(nw)[None, None, :]).reshape(n, 16 * nw)
    X = x.astype(np.float64)
    V = np.einsum("nd,nkd->nk", X, X[cand])  # exact fp64 dots
    V = np.maximum(V, 0.0)
    top = np.argpartition(-V, K, axis=1)[:, :K]
    rows = np.arange(n)[:, None]
    v = V[rows, top]
    cols = cand[rows, top]
    m = v.max(axis=1, keepdims=True)
    ex = np.exp(v - m)
    Dm = ex.sum(axis=1, keepdims=True) + (N - K) * np.exp(-m)
    base = (np.exp(-m) / Dm).astype(np.float32)
    kept = (ex / Dm).astype(np.float32)
    out = np.empty((n, N), np.float32)
    out[:] = base
    out[rows, cols] = kept
    return out


_CACHED_NC = None


def kernel(node_emb):
    global _CACHED_NC
    from concourse.bass_utils import run_bass_kernel_spmd

    if _CACHED_NC is None:
        _CACHED_NC = build()
    x = np.asarray(node_emb, dtype=np.float32)
    in_maps = _prep_inputs(x)
    res = run_bass_kernel_spmd(_CACHED_NC, in_maps, core_ids=list(range(NCORES)))
    pooled = np.concatenate([res.results[c]["out"] for c in range(NCORES)], axis=0)
    return _host_finish(x, pooled)


# revision 15
# speedup vs baseline: 1.0807x; 1.0199x over previous
"""Trainium2 Bass kernel for AdaptiveEmbeddingGraphBuilder.

Computes out = row_softmax(topk_mask(relu(E @ E.T), k=10)) for E [8192, 64],
row-sharded across 8 NeuronCores (1024 rows each).

Device side (per core, per 128-row block of A = E_rows @ E_full^T):
  - PE: plain fp8(e4m3) matmuls, K=64, into eight 1024-wide PSUM regions
    covering the 8192 columns.  Measured: the PE streams 512 moving rows
    per matmul at a fixed ~427 ns (1.2 GHz, no pstate ramp; none of the
    fp8 perf modes change it), so PE time is pinned at ~56 us/core and is
    the critical path.  fp8 dot noise (~0.3) is irrelevant for window
    *ranking* (margins are >10) and the host recomputes exact values.
  - Regions alternate consumers: even regions go to ACT (fp32->fp16 copy
    into strip tile A16), odd regions are consumed by DVE fused
    scalar_tensor_tensor (single PSUM input, as the ISA requires):
      tblk[:, c] = max(psum_odd[:, j], A16_even[:, j]),  c = 1024*i + j
    so pooled col c = max(A[:, 2048 i + j], A[:, 2048 i + 1024 + j]),
    written as fp8 (6% value noise, fine for ranking).
  - DMA out pooled [128, 4096] fp8 per block in two halves.

Host side: per row take the top-16 pooled 2-column windows (any column
with value >= v10 lands in a window whose pooled value is >= v10, and at
most 10 windows can satisfy that, so top-16 always contains the true
top-10); recompute the 32 candidate dots exactly in fp64, take the exact
top-10, and emit the exact masked softmax (kept entries exp(v-m)/D,
dropped entries exp(-m)/D with D = sum exp(v_k-m) + (N-10) exp(-m)).

Measured: 9.4e-8 absmax-rel, 1.2e-5 visible-element-rel vs the jax
reference (fp16 matmul variant measured identically).
"""

import numpy as np

N = 8192
D = 64
K = 10
NCORES = 8
P = 128
REG = 1024  # PSUM region width (2 banks)
NREG = 8
MM = 512  # single-matmul moving width
ROWS_PER_CORE = N // NCORES  # 1024
NBLOCKS = ROWS_PER_CORE // P  # 8
ACCW = 4096  # pooled output width per row
KWIN = 16  # host-side windows rechecked per row


def build(n=N, rows_per_core=ROWS_PER_CORE):
    import concourse.bacc as bacc
    import concourse.mybir as mybir
    import concourse.tile as tile

    nblocks = rows_per_core // P
    f32 = mybir.dt.float32
    f16 = mybir.dt.float16
    f8 = mybir.dt.float8e4
    Copy = mybir.ActivationFunctionType.Copy
    Max = mybir.AluOpType.max
    nc = bacc.Bacc("TRN2", target_bir_lowering=False, debug=False)
    et_d = nc.declare_dram_parameter("et", [D, n], f8, isOutput=False)
    lhs_d = nc.declare_dram_parameter("lhs", [D, rows_per_core], f8, isOutput=False)
    out_d = nc.declare_dram_parameter("out", [rows_per_core, ACCW], f8, isOutput=True)

    with tile.TileContext(nc) as tc:
        with (
            tc.tile_pool(name="const", bufs=1) as cpool,
            tc.tile_pool(name="acc", bufs=3) as apool,
            tc.tile_pool(name="outp", bufs=3) as opool,
            tc.tile_pool(name="psum", bufs=4, space="PSUM") as ppool,
        ):
            lhs_sb = cpool.tile([D, rows_per_core], f8)
            et_sb = cpool.tile([D, n], f8)
            # fine-grained input DMAs: the first matmul needs only
            # lhs[:, :128] and et[:, :512]
            nc.sync.dma_start(out=lhs_sb[:, 0:P], in_=lhs_d[:, 0:P])
            nc.sync.dma_start(out=et_sb[:, 0:MM], in_=et_d[:, 0:MM])
            nc.sync.dma_start(
                out=lhs_sb[:, P:rows_per_core], in_=lhs_d[:, P:rows_per_core]
            )
            nc.sync.dma_start(out=et_sb[:, MM:REG], in_=et_d[:, MM:REG])
            for r in range(1, NREG):
                nc.sync.dma_start(
                    out=et_sb[:, r * REG : (r + 1) * REG],
                    in_=et_d[:, r * REG : (r + 1) * REG],
                )

            def region_matmuls(dst, b, r):
                for c in range(REG // MM):
                    lo = r * REG + c * MM
                    nc.tensor.matmul(
                        out=dst[:, c * MM : (c + 1) * MM],
                        lhsT=lhs_sb[:, b * P : (b + 1) * P],
                        rhs=et_sb[:, lo : lo + MM],
                        start=True,
                        stop=True,
                    )

            for b in range(nblocks):
                A16 = apool.tile([P, ACCW], f16, tag="A")
                tblk = opool.tile([P, ACCW], f8, tag="T")
                for i in range(4):
                    pa = ppool.tile([P, REG], f32, tag="ps")
                    region_matmuls(pa, b, 2 * i)
                    nc.scalar.activation(
                        out=A16[:, i * REG : (i + 1) * REG], in_=pa[:], func=Copy
                    )
                    pb = ppool.tile([P, REG], f32, tag="ps")
                    region_matmuls(pb, b, 2 * i + 1)
                    # fused PSUM read + fold with the ACT strip
                    nc.vector.scalar_tensor_tensor(
                        out=tblk[:, i * REG : (i + 1) * REG],
                        in0=pb[:],
                        scalar=-3.0e38,
                        in1=A16[:, i * REG : (i + 1) * REG],
                        op0=Max,
                        op1=Max,
                    )
                nc.sync.dma_start(out=out_d[b * P : (b + 1) * P, :], in_=tblk[:])
    nc.compile()
    return nc


def _prep_inputs(node_emb):
    """fp8 cast + transpose + row-shard. Returns per-core in_maps."""
    import ml_dtypes

    x = np.asarray(node_emb, dtype=np.float32)
    cat = x.astype(ml_dtypes.float8_e4m3)  # [n, 64]
    et = np.ascontiguousarray(cat.T)  # [64, n]
    in_maps = []
    for c in range(NCORES):
        lhs = np.ascontiguousarray(cat[c * ROWS_PER_CORE : (c + 1) * ROWS_PER_CORE].T)
        in_maps.append({"et": et, "lhs": lhs})
    return in_maps


def _host_finish(x, pooled):
    """Exact top-10 masked softmax from the pooled device output.

    x: [N, 64] fp32 node embeddings; pooled: [N, 4096] with
    pooled[:, 1024 i + j] = max(A[:, 2048 i + j], A[:, 2048 i + 1024 + j]).
    """
    Pv = pooled.astype(np.float32)
    n = Pv.shape[0]
    w = np.argpartition(-Pv, KWIN, axis=1)[:, :KWIN]  # [n,KWIN] top windows
    c0 = 2 * REG * (w // REG) + (w % REG)
    cand = np.stack([c0, c0 + REG], axis=2).reshape(n, 2 * KWIN)
    X = x.astype(np.float64)
    V = np.einsum("nd,nkd->nk", X, X[cand])  # exact fp64 dots
    V = np.maximum(V, 0.0)
    top = np.argpartition(-V, K, axis=1)[:, :K]
    rows = np.arange(n)[:, None]
    v = V[rows, top]
    cols = cand[rows, top]
    m = v.max(axis=1, keepdims=True)
    ex = np.exp(v - m)
    Dm = ex.sum(axis=1, keepdims=True) + (N - K) * np.exp(-m)
    base = (np.exp(-m) / Dm).astype(np.float32)
    kept = (ex / Dm).astype(np.float32)
    out = np.empty((n, N), np.float32)
    out[:] = base
    out[rows, cols] = kept
    return out


_CACHED_NC = None


def kernel(node_emb):
    global _CACHED_NC
    from concourse.bass_utils import run_bass_kernel_spmd

    if _CACHED_NC is None:
        _CACHED_NC = build()
    x = np.asarray(node_emb, dtype=np.float32)
    in_maps = _prep_inputs(x)
    res = run_bass_kernel_spmd(_CACHED_NC, in_maps, core_ids=list(range(NCORES)))
    pooled = np.concatenate([res.results[c]["out"] for c in range(NCORES)], axis=0)
    return _host_finish(x, pooled)


# revision 19
# speedup vs baseline: 1.1201x; 1.0364x over previous
"""Trainium2 Bass kernel for AdaptiveEmbeddingGraphBuilder.

Computes out = row_softmax(topk_mask(relu(E @ E.T), k=10)) for E [8192, 64],
row-sharded across 8 NeuronCores (1024 rows each).

Device side (per core, per 128-row block of A = E_rows @ E_full^T):
  - PE: plain fp8(e4m3) matmuls, K=64, into eight 1024-wide PSUM regions
    covering the 8192 columns.  Measured: the PE streams 512 moving rows
    per matmul at a fixed ~427 ns (1.2 GHz, no pstate ramp; none of the
    fp8 perf modes change it), so PE time is pinned at ~56 us/core and is
    the critical path.  fp8 dot noise (~0.3) is irrelevant for window
    *ranking* (margins are >10) and the host recomputes exact values.
  - Regions alternate consumers: even regions go to ACT (fp32->fp16 copy
    into strip tile A16), odd regions are consumed by DVE fused
    scalar_tensor_tensor (single PSUM input, as the ISA requires):
      tblk[:, c] = max(psum_odd[:, j], A16_even[:, j]),  c = 1024*i + j
    so pooled col c = max(A[:, 2048 i + j], A[:, 2048 i + 1024 + j]),
    written as fp8 (6% value noise, fine for ranking).
  - DMA out pooled [128, 4096] fp8 per block in two halves.

Host side: per row take the top-16 pooled 2-column windows (any column
with value >= v10 lands in a window whose pooled value is >= v10, and at
most 10 windows can satisfy that, so top-16 always contains the true
top-10); recompute the 32 candidate dots exactly in fp64, take the exact
top-10, and emit the exact masked softmax (kept entries exp(v-m)/D,
dropped entries exp(-m)/D with D = sum exp(v_k-m) + (N-10) exp(-m)).

Measured: 9.4e-8 absmax-rel, 1.2e-5 visible-element-rel vs the jax
reference (fp16 matmul variant measured identically).
"""

import numpy as np

N = 8192
D = 64
K = 10
NCORES = 8
P = 128
REG = 1024  # PSUM region width (2 banks)
NREG = 8
MM = 512  # single-matmul moving width
ROWS_PER_CORE = N // NCORES  # 1024
NBLOCKS = ROWS_PER_CORE // P  # 8
ACCW = 4096  # pooled output width per row
KWIN = 16  # host-side windows rechecked per row


def build(n=N, rows_per_core=ROWS_PER_CORE):
    import concourse.bacc as bacc
    import concourse.mybir as mybir
    import concourse.tile as tile

    nblocks = rows_per_core // P
    f32 = mybir.dt.float32
    f16 = mybir.dt.float16
    f8 = mybir.dt.float8e4
    Copy = mybir.ActivationFunctionType.Copy
    Max = mybir.AluOpType.max
    nc = bacc.Bacc("TRN2", target_bir_lowering=False, debug=False)
    # et is permuted per-core on the host: the core's own 1024 rows first,
    # so the stationary weights are et_sb[:, b*128:(b+1)*128] in one SPMD
    # program; the host un-permutes pooled columns afterward.
    et_d = nc.declare_dram_parameter("et", [D, n], f8, isOutput=False)
    out_d = nc.declare_dram_parameter("out", [rows_per_core, ACCW], f8, isOutput=True)

    with tile.TileContext(nc) as tc:
        with (
            tc.tile_pool(name="const", bufs=1) as cpool,
            tc.tile_pool(name="acc", bufs=3) as apool,
            tc.tile_pool(name="outp", bufs=3) as opool,
            tc.tile_pool(name="psum", bufs=4, space="PSUM") as ppool,
        ):
            et_sb = cpool.tile([D, n], f8)
            lhs_sb = et_sb  # stationary weights live in the permuted et
            # fine-grained input DMAs: the first matmul needs only et[:, :512]
            nc.sync.dma_start(out=et_sb[:, 0:MM], in_=et_d[:, 0:MM])
            nc.sync.dma_start(out=et_sb[:, MM:REG], in_=et_d[:, MM:REG])
            for r in range(1, NREG):
                nc.sync.dma_start(
                    out=et_sb[:, r * REG : (r + 1) * REG],
                    in_=et_d[:, r * REG : (r + 1) * REG],
                )

            def region_matmuls(dst, b, r):
                for c in range(REG // MM):
                    lo = r * REG + c * MM
                    nc.tensor.matmul(
                        out=dst[:, c * MM : (c + 1) * MM],
                        lhsT=lhs_sb[:, b * P : (b + 1) * P],
                        rhs=et_sb[:, lo : lo + MM],
                        start=True,
                        stop=True,
                    )

            for b in range(nblocks):
                A16 = apool.tile([P, ACCW], f16, tag="A")
                tblk = opool.tile([P, ACCW], f8, tag="T")
                for i in range(4):
                    pa = ppool.tile([P, REG], f32, tag="ps")
                    region_matmuls(pa, b, 2 * i)
                    nc.scalar.activation(
                        out=A16[:, i * REG : (i + 1) * REG], in_=pa[:], func=Copy
                    )
                    pb = ppool.tile([P, REG], f32, tag="ps")
                    region_matmuls(pb, b, 2 * i + 1)
                    # fused PSUM read + fold with the ACT strip
                    nc.vector.scalar_tensor_tensor(
                        out=tblk[:, i * REG : (i + 1) * REG],
                        in0=pb[:],
                        scalar=-3.0e38,
                        in1=A16[:, i * REG : (i + 1) * REG],
                        op0=Max,
                        op1=Max,
                    )
                nc.sync.dma_start(out=out_d[b * P : (b + 1) * P, :], in_=tblk[:])
    nc.compile()
    return nc


def _core_perm(c):
    """Device column -> global column order for core c (own rows first)."""
    own = np.arange(c * ROWS_PER_CORE, (c + 1) * ROWS_PER_CORE)
    rest = np.concatenate(
        [np.arange(0, c * ROWS_PER_CORE), np.arange((c + 1) * ROWS_PER_CORE, N)]
    )
    return np.concatenate([own, rest])


def _prep_inputs(node_emb):
    """fp8 cast + transpose + per-core column permutation (own rows first)."""
    import ml_dtypes

    x = np.asarray(node_emb, dtype=np.float32)
    cat = x.astype(ml_dtypes.float8_e4m3)  # [n, 64]
    et = np.ascontiguousarray(cat.T)  # [64, n]
    in_maps = []
    for c in range(NCORES):
        in_maps.append({"et": np.ascontiguousarray(et[:, _core_perm(c)])})
    return in_maps


def _host_finish(x, pooled):
    """Exact top-10 masked softmax from the pooled device output.

    x: [N, 64] fp32 node embeddings; pooled: [N, 4096] with
    pooled[:, 1024 i + j] = max(A[:, 2048 i + j], A[:, 2048 i + 1024 + j]).
    """
    Pv = pooled.astype(np.float32)
    n = Pv.shape[0]
    w = np.argpartition(-Pv, KWIN, axis=1)[:, :KWIN]  # [n,KWIN] top windows
    c0 = 2 * REG * (w // REG) + (w % REG)
    cand = np.stack([c0, c0 + REG], axis=2).reshape(n, 2 * KWIN)
    # device columns -> global columns (per-core permutation)
    perms = np.stack([_core_perm(c) for c in range(NCORES)])  # [NCORES, N]
    cand = perms[np.arange(n)[:, None] // ROWS_PER_CORE, cand]
    X = x.astype(np.float64)
    V = np.einsum("nd,nkd->nk", X, X[cand])  # exact fp64 dots
    V = np.maximum(V, 0.0)
    top = np.argpartition(-V, K, axis=1)[:, :K]
    rows = np.arange(n)[:, None]
    v = V[rows, top]
    cols = cand[rows, top]
    m = v.max(axis=1, keepdims=True)
    ex = np.exp(v - m)
    Dm = ex.sum(axis=1, keepdims=True) + (N - K) * np.exp(-m)
    base = (np.exp(-m) / Dm).astype(np.float32)
    kept = (ex / Dm).astype(np.float32)
    out = np.empty((n, N), np.float32)
    out[:] = base
    out[rows, cols] = kept
    return out


_CACHED_NC = None


def kernel(node_emb):
    global _CACHED_NC
    from concourse.bass_utils import run_bass_kernel_spmd

    if _CACHED_NC is None:
        _CACHED_NC = build()
    x = np.asarray(node_emb, dtype=np.float32)
    in_maps = _prep_inputs(x)
    res = run_bass_kernel_spmd(_CACHED_NC, in_maps, core_ids=list(range(NCORES)))
    pooled = np.concatenate([res.results[c]["out"] for c in range(NCORES)], axis=0)
    return _host_finish(x, pooled)


# revision 21
# speedup vs baseline: 1.1298x; 1.0086x over previous
"""Trainium2 Bass kernel for AdaptiveEmbeddingGraphBuilder.

Computes out = row_softmax(topk_mask(relu(E @ E.T), k=10)) for E [8192, 64],
row-sharded across 8 NeuronCores (1024 rows each).

Device side (per core, per 128-row block of A = E_rows @ E_full^T):
  - PE: plain fp8(e4m3) matmuls, K=64, into eight 1024-wide PSUM regions
    covering the 8192 columns.  Measured: the PE streams 512 moving rows
    per matmul at a fixed ~427 ns (1.2 GHz, no pstate ramp; none of the
    fp8 perf modes change it), so PE time is pinned at ~56 us/core and is
    the critical path.  fp8 dot noise (~0.3) is irrelevant for window
    *ranking* (margins are >10) and the host recomputes exact values.
  - Regions alternate consumers: even regions go to ACT (fp32->fp16 copy
    into strip tile A16), odd regions are consumed by DVE fused
    scalar_tensor_tensor (single PSUM input, as the ISA requires):
      tblk[:, c] = max(psum_odd[:, j], A16_even[:, j]),  c = 1024*i + j
    so pooled col c = max(A[:, 2048 i + j], A[:, 2048 i + 1024 + j]),
    written as fp8 (6% value noise, fine for ranking).
  - DMA out pooled [128, 4096] fp8 per block in two halves.

Host side: per row take the top-16 pooled 2-column windows (any column
with value >= v10 lands in a window whose pooled value is >= v10, and at
most 10 windows can satisfy that, so top-16 always contains the true
top-10); recompute the 32 candidate dots exactly in fp64, take the exact
top-10, and emit the exact masked softmax (kept entries exp(v-m)/D,
dropped entries exp(-m)/D with D = sum exp(v_k-m) + (N-10) exp(-m)).

Measured: 9.4e-8 absmax-rel, 1.2e-5 visible-element-rel vs the jax
reference (fp16 matmul variant measured identically).
"""

import numpy as np

N = 8192
D = 64
K = 10
NCORES = 8
P = 128
REG = 1024  # PSUM region width (2 banks)
NREG = 8
MM = 512  # single-matmul moving width
ROWS_PER_CORE = N // NCORES  # 1024
NBLOCKS = ROWS_PER_CORE // P  # 8
ACCW = 4096  # pooled output width per row
KWIN = 16  # host-side windows rechecked per row


def build(n=N, rows_per_core=ROWS_PER_CORE):
    import concourse.bacc as bacc
    import concourse.mybir as mybir
    import concourse.tile as tile

    nblocks = rows_per_core // P
    f32 = mybir.dt.float32
    f16 = mybir.dt.float16
    f8 = mybir.dt.float8e4
    Copy = mybir.ActivationFunctionType.Copy
    Max = mybir.AluOpType.max
    nc = bacc.Bacc("TRN2", target_bir_lowering=False, debug=False)
    # et is permuted per-core on the host: the core's own 1024 rows first,
    # so the stationary weights are et_sb[:, b*128:(b+1)*128] in one SPMD
    # program; the host un-permutes pooled columns afterward.
    et_d = nc.declare_dram_parameter("et", [D, n], f8, isOutput=False)
    out_d = nc.declare_dram_parameter("out", [rows_per_core, ACCW], f8, isOutput=True)

    with tile.TileContext(nc) as tc:
        with (
            tc.tile_pool(name="const", bufs=1) as cpool,
            tc.tile_pool(name="acc", bufs=3) as apool,
            tc.tile_pool(name="outp", bufs=3) as opool,
            tc.tile_pool(name="psum", bufs=4, space="PSUM") as ppool,
        ):
            et_sb = cpool.tile([D, n], f8)
            lhs_sb = et_sb  # stationary weights live in the permuted et
            # fine-grained input DMAs: the first matmul needs only et[:, :512]
            nc.sync.dma_start(out=et_sb[:, 0:MM], in_=et_d[:, 0:MM])
            nc.sync.dma_start(out=et_sb[:, MM:REG], in_=et_d[:, MM:REG])
            for r in range(1, NREG):
                nc.sync.dma_start(
                    out=et_sb[:, r * REG : (r + 1) * REG],
                    in_=et_d[:, r * REG : (r + 1) * REG],
                )

            def region_matmuls(dst, b, r):
                for c in range(REG // MM):
                    lo = r * REG + c * MM
                    nc.tensor.matmul(
                        out=dst[:, c * MM : (c + 1) * MM],
                        lhsT=lhs_sb[:, b * P : (b + 1) * P],
                        rhs=et_sb[:, lo : lo + MM],
                        start=True,
                        stop=True,
                    )

            for b in range(nblocks):
                A16 = apool.tile([P, ACCW], f16, tag="A")
                tblk0 = opool.tile([P, 2 * REG], f8, tag="T0")
                tblk1 = opool.tile([P, 2 * REG], f8, tag="T1")
                halves = [tblk0, tblk1]
                for i in range(4):
                    pa = ppool.tile([P, REG], f32, tag="ps")
                    region_matmuls(pa, b, 2 * i)
                    nc.scalar.activation(
                        out=A16[:, i * REG : (i + 1) * REG], in_=pa[:], func=Copy
                    )
                    pb = ppool.tile([P, REG], f32, tag="ps")
                    region_matmuls(pb, b, 2 * i + 1)
                    # fused PSUM read + fold with the ACT strip
                    nc.vector.scalar_tensor_tensor(
                        out=halves[i // 2][:, (i % 2) * REG : (i % 2 + 1) * REG],
                        in0=pb[:],
                        scalar=-3.0e38,
                        in1=A16[:, i * REG : (i + 1) * REG],
                        op0=Max,
                        op1=Max,
                    )
                    if i % 2 == 1:
                        h = i // 2
                        nc.sync.dma_start(
                            out=out_d[b * P : (b + 1) * P, 2 * h * REG : 2 * (h + 1) * REG],
                            in_=halves[h][:],
                        )
    nc.compile()
    return nc


def _core_perm(c):
    """Device column -> global column order for core c (own rows first)."""
    own = np.arange(c * ROWS_PER_CORE, (c + 1) * ROWS_PER_CORE)
    rest = np.concatenate(
        [np.arange(0, c * ROWS_PER_CORE), np.arange((c + 1) * ROWS_PER_CORE, N)]
    )
    return np.concatenate([own, rest])


def _prep_inputs(node_emb):
    """fp8 cast + transpose + per-core column permutation (own rows first)."""
    import ml_dtypes

    x = np.asarray(node_emb, dtype=np.float32)
    cat = x.astype(ml_dtypes.float8_e4m3)  # [n, 64]
    et = np.ascontiguousarray(cat.T)  # [64, n]
    in_maps = []
    for c in range(NCORES):
        in_maps.append({"et": np.ascontiguousarray(et[:, _core_perm(c)])})
    return in_maps


def _host_finish(x, pooled):
    """Exact top-10 masked softmax from the pooled device output.

    x: [N, 64] fp32 node embeddings; pooled: [N, 4096] with
    pooled[:, 1024 i + j] = max(A[:, 2048 i + j], A[:, 2048 i + 1024 + j]).
    """
    Pv = pooled.astype(np.float32)
    n = Pv.shape[0]
    w = np.argpartition(-Pv, KWIN, axis=1)[:, :KWIN]  # [n,KWIN] top windows
    c0 = 2 * REG * (w // REG) + (w % REG)
    cand = np.stack([c0, c0 + REG], axis=2).reshape(n, 2 * KWIN)
    # device columns -> global columns (per-core permutation)
    perms = np.stack([_core_perm(c) for c in range(NCORES)])  # [NCORES, N]
    cand = perms[np.arange(n)[:, None] // ROWS_PER_CORE, cand]
    X = x.astype(np.float64)
    V = np.einsum("nd,nkd->nk", X, X[cand])  # exact fp64 dots
    V = np.maximum(V, 0.0)
    top = np.argpartition(-V, K, axis=1)[:, :K]
    rows = np.arange(n)[:, None]
    v = V[rows, top]
    cols = cand[rows, top]
    m = v.max(axis=1, keepdims=True)
    ex = np.exp(v - m)
    Dm = ex.sum(axis=1, keepdims=True) + (N - K) * np.exp(-m)
    base = (np.exp(-m) / Dm).astype(np.float32)
    kept = (ex / Dm).astype(np.float32)
    out = np.empty((n, N), np.float32)
    out[:] = base
    out[rows, cols] = kept
    return out


_CACHED_NC = None


def kernel(node_emb):
    global _CACHED_NC
    from concourse.bass_utils import run_bass_kernel_spmd

    if _CACHED_NC is None:
        _CACHED_NC = build()
    x = np.asarray(node_emb, dtype=np.float32)
    in_maps = _prep_inputs(x)
    res = run_bass_kernel_spmd(_CACHED_NC, in_maps, core_ids=list(range(NCORES)))
    pooled = np.concatenate([res.results[c]["out"] for c in range(NCORES)], axis=0)
    return _host_finish(x, pooled)


# revision 22
# speedup vs baseline: 1.1405x; 1.0095x over previous
"""Trainium2 Bass kernel for AdaptiveEmbeddingGraphBuilder.

Computes out = row_softmax(topk_mask(relu(E @ E.T), k=10)) for E [8192, 64],
row-sharded across 8 NeuronCores (1024 rows each).

Device side (per core, per 128-row block of A = E_rows @ E_full^T):
  - PE: plain fp8(e4m3) matmuls, K=64, into eight 1024-wide PSUM regions
    covering the 8192 columns.  Measured: the PE streams 512 moving rows
    per matmul at a fixed ~427 ns (1.2 GHz, no pstate ramp; none of the
    fp8 perf modes change it), so PE time is pinned at ~56 us/core and is
    the critical path.  fp8 dot noise (~0.3) is irrelevant for window
    *ranking* (margins are >10) and the host recomputes exact values.
  - Regions alternate consumers: even regions go to ACT (fp32->fp16 copy
    into strip tile A16), odd regions are consumed by DVE fused
    scalar_tensor_tensor (single PSUM input, as the ISA requires):
      tblk[:, c] = max(psum_odd[:, j], A16_even[:, j]),  c = 1024*i + j
    so pooled col c = max(A[:, 2048 i + j], A[:, 2048 i + 1024 + j]),
    written as fp8 (6% value noise, fine for ranking).
  - DMA out pooled [128, 4096] fp8 per block in two halves.

Host side: per row take the top-16 pooled 2-column windows (any column
with value >= v10 lands in a window whose pooled value is >= v10, and at
most 10 windows can satisfy that, so top-16 always contains the true
top-10); recompute the 32 candidate dots exactly in fp64, take the exact
top-10, and emit the exact masked softmax (kept entries exp(v-m)/D,
dropped entries exp(-m)/D with D = sum exp(v_k-m) + (N-10) exp(-m)).

Measured: 9.4e-8 absmax-rel, 1.2e-5 visible-element-rel vs the jax
reference (fp16 matmul variant measured identically).
"""

import numpy as np

N = 8192
D = 64
K = 10
NCORES = 8
P = 128
REG = 1024  # PSUM region width (2 banks)
NREG = 8
MM = 512  # single-matmul moving width
ROWS_PER_CORE = N // NCORES  # 1024
NBLOCKS = ROWS_PER_CORE // P  # 8
ACCW = 4096  # pooled output width per row
KWIN = 16  # host-side windows rechecked per row


def build(n=N, rows_per_core=ROWS_PER_CORE):
    import concourse.bacc as bacc
    import concourse.mybir as mybir
    import concourse.tile as tile

    nblocks = rows_per_core // P
    f32 = mybir.dt.float32
    f16 = mybir.dt.float16
    f8 = mybir.dt.float8e4
    Copy = mybir.ActivationFunctionType.Copy
    Max = mybir.AluOpType.max
    nc = bacc.Bacc("TRN2", target_bir_lowering=False, debug=False)
    # et is permuted per-core on the host: the core's own 1024 rows first,
    # so the stationary weights are et_sb[:, b*128:(b+1)*128] in one SPMD
    # program; the host un-permutes pooled columns afterward.
    et_d = nc.declare_dram_parameter("et", [D, n], f8, isOutput=False)
    out_d = nc.declare_dram_parameter("out", [rows_per_core, ACCW], f8, isOutput=True)

    with tile.TileContext(nc) as tc:
        with (
            tc.tile_pool(name="const", bufs=1) as cpool,
            tc.tile_pool(name="acc", bufs=3) as apool,
            tc.tile_pool(name="outp", bufs=3) as opool,
            tc.tile_pool(name="psum", bufs=4, space="PSUM") as ppool,
        ):
            et_sb = cpool.tile([D, n], f8)
            lhs_sb = et_sb  # stationary weights live in the permuted et
            # fine-grained input DMAs: the first matmul needs only et[:, :512]
            nc.sync.dma_start(out=et_sb[:, 0:MM], in_=et_d[:, 0:MM], single_packet=True)
            nc.sync.dma_start(out=et_sb[:, MM:REG], in_=et_d[:, MM:REG])
            for r in range(1, NREG):
                nc.sync.dma_start(
                    out=et_sb[:, r * REG : (r + 1) * REG],
                    in_=et_d[:, r * REG : (r + 1) * REG],
                )

            def region_matmuls(dst, b, r):
                for c in range(REG // MM):
                    lo = r * REG + c * MM
                    nc.tensor.matmul(
                        out=dst[:, c * MM : (c + 1) * MM],
                        lhsT=lhs_sb[:, b * P : (b + 1) * P],
                        rhs=et_sb[:, lo : lo + MM],
                        start=True,
                        stop=True,
                    )

            for b in range(nblocks):
                A16 = apool.tile([P, ACCW], f16, tag="A")
                tblk0 = opool.tile([P, 2 * REG], f8, tag="T0")
                tblk1 = opool.tile([P, 2 * REG], f8, tag="T1")
                halves = [tblk0, tblk1]
                for i in range(4):
                    pa = ppool.tile([P, REG], f32, tag="ps")
                    region_matmuls(pa, b, 2 * i)
                    nc.scalar.activation(
                        out=A16[:, i * REG : (i + 1) * REG], in_=pa[:], func=Copy
                    )
                    pb = ppool.tile([P, REG], f32, tag="ps")
                    region_matmuls(pb, b, 2 * i + 1)
                    # fused PSUM read + fold with the ACT strip
                    nc.vector.scalar_tensor_tensor(
                        out=halves[i // 2][:, (i % 2) * REG : (i % 2 + 1) * REG],
                        in0=pb[:],
                        scalar=-3.0e38,
                        in1=A16[:, i * REG : (i + 1) * REG],
                        op0=Max,
                        op1=Max,
                    )
                    if i % 2 == 1:
                        h = i // 2
                        nc.sync.dma_start(
                            out=out_d[b * P : (b + 1) * P, 2 * h * REG : 2 * (h + 1) * REG],
                            in_=halves[h][:],
                        )
    nc.compile()
    return nc


def _core_perm(c):
    """Device column -> global column order for core c (own rows first)."""
    own = np.arange(c * ROWS_PER_CORE, (c + 1) * ROWS_PER_CORE)
    rest = np.concatenate(
        [np.arange(0, c * ROWS_PER_CORE), np.arange((c + 1) * ROWS_PER_CORE, N)]
    )
    return np.concatenate([own, rest])


def _prep_inputs(node_emb):
    """fp8 cast + transpose + per-core column permutation (own rows first)."""
    import ml_dtypes

    x = np.asarray(node_emb, dtype=np.float32)
    cat = x.astype(ml_dtypes.float8_e4m3)  # [n, 64]
    et = np.ascontiguousarray(cat.T)  # [64, n]
    in_maps = []
    for c in range(NCORES):
        in_maps.append({"et": np.ascontiguousarray(et[:, _core_perm(c)])})
    return in_maps


def _host_finish(x, pooled):
    """Exact top-10 masked softmax from the pooled device output.

    x: [N, 64] fp32 node embeddings; pooled: [N, 4096] with
    pooled[:, 1024 i + j] = max(A[:, 2048 i + j], A[:, 2048 i + 1024 + j]).
    """
    Pv = pooled.astype(np.float32)
    n = Pv.shape[0]
    w = np.argpartition(-Pv, KWIN, axis=1)[:, :KWIN]  # [n,KWIN] top windows
    c0 = 2 * REG * (w // REG) + (w % REG)
    cand = np.stack([c0, c0 + REG], axis=2).reshape(n, 2 * KWIN)
    # device columns -> global columns (per-core permutation)
    perms = np.stack([_core_perm(c) for c in range(NCORES)])  # [NCORES, N]
    cand = perms[np.arange(n)[:, None] // ROWS_PER_CORE, cand]
    X = x.astype(np.float64)
    V = np.einsum("nd,nkd->nk", X, X[cand])  # exact fp64 dots
    V = np.maximum(V, 0.0)
    top = np.argpartition(-V, K, axis=1)[:, :K]
    rows = np.arange(n)[:, None]
    v = V[rows, top]
    cols = cand[rows, top]
    m = v.max(axis=1, keepdims=True)
    ex = np.exp(v - m)
    Dm = ex.sum(axis=1, keepdims=True) + (N - K) * np.exp(-m)
    base = (np.exp(-m) / Dm).astype(np.float32)
    kept = (ex / Dm).astype(np.float32)
    out = np.empty((n, N), np.float32)
    out[:] = base
    out[rows, cols] = kept
    return out


_CACHED_NC = None


def kernel(node_emb):
    global _CACHED_NC
    from concourse.bass_utils import run_bass_kernel_spmd

    if _CACHED_NC is None:
        _CACHED_NC = build()
    x = np.asarray(node_emb, dtype=np.float32)
    in_maps = _prep_inputs(x)
    res = run_bass_kernel_spmd(_CACHED_NC, in_maps, core_ids=list(range(NCORES)))
    pooled = np.concatenate([res.results[c]["out"] for c in range(NCORES)], axis=0)
    return _host_finish(x, pooled)
